# revision 14
# baseline (speedup 1.0000x reference)
"""MoE MLP (2 experts, token-type routing) on 8 TRN2 NeuronCores.

Strategy:
  - Host routes tokens by type: type-0 tokens -> cores 0-3 (expert S),
    type-1 tokens -> cores 4-7 (expert L). Each core gets the same static
    token count T (padded), so one SPMD NEFF serves all 8 cores; the
    expert selection is purely which weight tensors each core receives.
  - Everything on-device is computed feature-major ("transposed"): both
    GEMMs take the natural weight layout as the stationary operand and
    tokens as the moving free dimension, so no transposes are needed
    anywhere. Per token chunk (<=512 tokens):
        H^T[h, t]  = gelu(W1[c, h]^T-contract x^T[c, t] + b1[h])   (ACT epilogue)
        Y^T[o, t]  = W2[h, o]-contract H^T[h, t] + b2[o]           (DVE epilogue)
  - bf16 matmuls with fp32 PSUM accumulation (~3e-3 scale-relative err).
  - Both expert weight matrices stay resident in SBUF (128KB/partition).
  - PE warmup matmuls on a zero tile run during the initial DMAs so the
    first real matmul executes at the warm 2.4 GHz clock; the first W1
    piece, the biases, and the chunk-0 x DMA are front-loaded so GEMM1
    starts ~8us in and the PE never stalls afterwards.
"""

import ml_dtypes
import numpy as np

C = 1024  # model dim
H = 4096  # hidden dim
P = 128  # partitions
KC = C // P  # 8  k-tiles for GEMM1 contraction
KH = H // P  # 32 k-tiles for GEMM2 contraction / h-tiles of GEMM1 output
MO = C // P  # 8  output-channel tiles
NT_MAX = 512  # max token chunk (matmul moving free dim)
N_CORES = 8

BF16 = ml_dtypes.bfloat16

_PROGRAM_CACHE: dict[tuple, object] = {}
last_results = None  # BassKernelResults of the most recent run (for profiling)


def _chunk_sizes(T0: int) -> tuple[int, ...]:
    """Split T0 tokens into near-equal chunks of <=512, each a multiple of 8.

    Equal-ish chunks keep the matmul moving dim large everywhere (so
    LDWEIGHTS stays hidden behind the matmul stream) instead of leaving a
    tiny remainder chunk.
    """
    T0 = max(T0, 32)
    n_chunks = -(-T0 // NT_MAX)
    base = -(-T0 // (n_chunks * 8)) * 8
    rest = T0 - base * (n_chunks - 1)
    last = max(32, -(-rest // 8) * 8)
    return tuple([base] * (n_chunks - 1) + [last])


def _build_program(chunks: tuple[int, ...]):
    import concourse.mybir as mybir
    import concourse.tile as tile
    from concourse import bacc

    T = sum(chunks)
    nc = bacc.Bacc("TRN2", target_bir_lowering=False, debug=False, num_devices=N_CORES)

    xt = nc.dram_tensor("xt", [C, T], mybir.dt.bfloat16, kind="ExternalInput").ap()
    w1 = nc.dram_tensor("w1", [C, H], mybir.dt.bfloat16, kind="ExternalInput").ap()
    w2 = nc.dram_tensor("w2", [H, C], mybir.dt.bfloat16, kind="ExternalInput").ap()
    b1 = nc.dram_tensor("b1", [P, KH], mybir.dt.float32, kind="ExternalInput").ap()
    b2 = nc.dram_tensor("b2", [P, MO], mybir.dt.float32, kind="ExternalInput").ap()
    yt = nc.dram_tensor("yt", [C, T], mybir.dt.float32, kind="ExternalOutput").ap()

    xt_r = xt.rearrange("(ko p) t -> p ko t", p=P)
    w1_r = w1.rearrange("(ko p) h -> p ko h", p=P)
    w2_r = w2.rearrange("(ko p) c -> p ko c", p=P)
    yt_r = yt.rearrange("(mo p) t -> p mo t", p=P)

    offs = [0]
    for ntc in chunks:
        offs.append(offs[-1] + ntc)

    with tile.TileContext(nc) as tc:
        with (
            tc.tile_pool(name="weights", bufs=1) as wpool,
            tc.tile_pool(name="xin", bufs=2) as xpool,
            tc.tile_pool(name="hbuf", bufs=1) as hpool,
            tc.tile_pool(name="obuf", bufs=1) as opool,
            tc.tile_pool(name="psum", bufs=8, space="PSUM") as pspool,
        ):
            # --- PE warmup: ~7us of dummy matmuls on a zero tile so HAM
            # un-throttles the PE clock before the first real matmul.
            warm_sb = wpool.tile([P, NT_MAX], mybir.dt.bfloat16, name="warm_sb")
            nc.vector.memset(warm_sb[:], 0.0)
            warm_ps = pspool.tile([P, NT_MAX], mybir.dt.float32, tag="ps", name="warm_ps")
            for _ in range(16):
                nc.tensor.matmul(
                    warm_ps[:], warm_sb[:, :P], warm_sb[:], start=True, stop=True
                )

            x_tiles = {}

            def load_x(ci):
                ntc = chunks[ci]
                t = xpool.tile([P, KC, ntc], mybir.dt.bfloat16, tag="x", name="x_sb")
                nc.sync.dma_start(t[:], xt_r[:, :, offs[ci] : offs[ci] + ntc])
                return t

            # DMA order is chosen for the startup critical path (the model's
            # DMA engines drain transfers roughly in issue order):
            #   w1 piece 0 -> biases (gelu epilogue releases PSUM slots; a
            #   late b1 stalls the PE via slot back-pressure) -> chunk-0 x ->
            #   rest of W1 -> W2.
            # W1 lives in 8 SEPARATE tiles (Tile tracks DMA deps per tile,
            # not per slice) so GEMM1 can start after ~1/8 of W1 landed.
            # Piece hh covers h-tiles j in [hh*4, hh*4+4).
            W1_PIECE = H // 8
            w1_sbs = []

            def load_w1_piece(hh):
                w1_piece = wpool.tile(
                    [P, KC, W1_PIECE], mybir.dt.bfloat16, name=f"w1_sb{hh}"
                )
                nc.sync.dma_start(
                    w1_piece[:], w1_r[:, :, hh * W1_PIECE : (hh + 1) * W1_PIECE]
                )
                w1_sbs.append(w1_piece)

            load_w1_piece(0)
            b1_sb = wpool.tile([P, KH], mybir.dt.float32, name="b1_sb")
            nc.sync.dma_start(b1_sb[:], b1[:])
            b2_sb = wpool.tile([P, MO], mybir.dt.float32, name="b2_sb")
            nc.sync.dma_start(b2_sb[:], b2[:])

            # chunk-0 activations: they gate the very first matmul
            x_tiles[0] = load_x(0)

            for hh in range(1, 8):
                load_w1_piece(hh)
            # W2 as 4 separate tiles; piece mm covers m-tiles in [mm*2, mm*2+2)
            W2_PIECE = C // 4
            w2_sbs = []
            for mm in range(4):
                w2_piece = wpool.tile(
                    [P, KH, W2_PIECE], mybir.dt.bfloat16, name=f"w2_sb{mm}"
                )
                nc.sync.dma_start(
                    w2_piece[:], w2_r[:, :, mm * W2_PIECE : (mm + 1) * W2_PIECE]
                )
                w2_sbs.append(w2_piece)

            for ci, nt in enumerate(chunks):
                x_sb = x_tiles.pop(ci) if ci in x_tiles else load_x(ci)

                # GEMM1: H^T tile j = sum_k W1[k-tile, j-tile].T @ X^T[k-tile]
                h_sb = hpool.tile([P, KH, nt], mybir.dt.bfloat16, tag="h", name="h_sb")
                for j in range(KH):
                    w1_piece = w1_sbs[j // 4]
                    jcol = (j % 4) * P
                    ps = pspool.tile([P, nt], mybir.dt.float32, tag="ps", name="ps")
                    for k in range(KC):
                        nc.tensor.matmul(
                            ps[:],
                            w1_piece[:, k, jcol : jcol + P],
                            x_sb[:, k, :],
                            start=(k == 0),
                            stop=(k == KC - 1),
                        )
                    # h = gelu(psum + b1) with bf16 downcast, fused on ACT
                    nc.scalar.activation(
                        h_sb[:, j, :],
                        ps[:],
                        mybir.ActivationFunctionType.Gelu,
                        bias=b1_sb[:, j : j + 1],
                        scale=1.0,
                    )

                # GEMM2: Y^T tile m = sum_k2 W2[k2-tile, m-tile].T @ H^T[k2-tile]
                o_sb = opool.tile([P, MO, nt], mybir.dt.float32, tag="o", name="o_sb")
                for m in range(MO):
                    w2_piece = w2_sbs[m // 2]
                    mcol = (m % 2) * P
                    ps2 = pspool.tile([P, nt], mybir.dt.float32, tag="ps", name="ps2")
                    for k2 in range(KH):
                        nc.tensor.matmul(
                            ps2[:],
                            w2_piece[:, k2, mcol : mcol + P],
                            h_sb[:, k2, :],
                            start=(k2 == 0),
                            stop=(k2 == KH - 1),
                        )
                    nc.vector.tensor_scalar_add(
                        o_sb[:, m, :], ps2[:], b2_sb[:, m : m + 1]
                    )
                    # per-m store: earlier m-tiles stream out while later m
                    # compute; matters for the kernel tail on the last chunk
                    nc.sync.dma_start(
                        yt_r[:, m, offs[ci] : offs[ci] + nt], o_sb[:, m, :]
                    )

    nc.compile()
    return nc


def kernel(x, token_types, w1_s, b1_s, w2_s, b2_s, w1_l, b1_l, w2_l, b2_l):
    global last_results
    from concourse.bass_utils import run_bass_kernel_spmd

    x = np.asarray(x, dtype=np.float32)
    tt = np.asarray(token_types).reshape(-1)
    B, N, Cin = x.shape
    assert Cin == C
    x_flat = x.reshape(-1, C)
    n_tok = x_flat.shape[0]

    idx0 = np.flatnonzero(tt == 0)
    idx1 = np.flatnonzero(tt == 1)
    half = N_CORES // 2
    per_core = max(
        (len(idx0) + half - 1) // half, (len(idx1) + half - 1) // half, 32
    )
    chunks = _chunk_sizes(per_core)
    T = sum(chunks)

    nc = _PROGRAM_CACHE.get(chunks)
    if nc is None:
        nc = _build_program(chunks)
        _PROGRAM_CACHE[chunks] = nc

    def stripe_bias(b):
        # b[KH*P] -> [P, KH] with b_sb[p, j] = b[j*P + p]
        b = np.asarray(b, dtype=np.float32)
        return np.ascontiguousarray(b.reshape(-1, P).T)

    experts = [
        (idx0, np.asarray(w1_s).astype(BF16), stripe_bias(b1_s),
         np.asarray(w2_s).astype(BF16), stripe_bias(b2_s)),
        (idx1, np.asarray(w1_l).astype(BF16), stripe_bias(b1_l),
         np.asarray(w2_l).astype(BF16), stripe_bias(b2_l)),
    ]

    in_maps = []
    core_slices = []  # index array per core
    for core in range(N_CORES):
        e = core // half
        idx, w1b, b1b, w2b, b2b = experts[e]
        lo = (core % half) * T
        sl = idx[lo : lo + T]
        core_slices.append(sl)
        ind = np.zeros(T, dtype=np.int64)
        ind[: len(sl)] = sl
        xt = np.ascontiguousarray(x_flat[ind].T).astype(BF16)  # [C, T]
        in_maps.append({"xt": xt, "w1": w1b, "b1": b1b, "w2": w2b, "b2": b2b})

    try:
        last_results = run_bass_kernel_spmd(nc, in_maps, core_ids=list(range(N_CORES)))
    except Exception:
        # transient NRT/device hiccups have been observed to clear on retry
        import time as _time

        _time.sleep(5)
        last_results = run_bass_kernel_spmd(nc, in_maps, core_ids=list(range(N_CORES)))

    out = np.zeros((n_tok, C), dtype=np.float32)
    for core in range(N_CORES):
        sl = core_slices[core]
        if len(sl):
            out[sl] = last_results.results[core]["yt"][:, : len(sl)].T
    return out.reshape(B, N, C)
